# revision 3
# baseline (speedup 1.0000x reference)
"""Trainium2 Bass kernel for nn_Ext_entLayer (temporal-GNN message passing).

Strategy (edge-parallel, collective-free):
  - Host sorts edges by (dst, inv). Cores own contiguous node ranges of
    N/8 nodes; every edge lands on the core owning its dst node, so the
    mean-aggregation is core-local (no cross-core reduction).
  - Device phase 1, per 640-edge "supertile" (5 subtiles of 128 edges):
      * ent_emb rows are gathered per subtile via a one-index-per-
        partition indirect DMA (the only indirect form TRN2 supports),
      * rel/time need no gather: per-subtile 0/1 one-hots feed
        transposed-histogram matmuls HT[r, pair] += oh_rel.T @ oh_pair,
        and the pair aggregate is recovered as HT.T-free transform
        matmuls against the SBUF-resident (padded) rel/time tables,
      * all edges of a (dst, inv) pair share dst, hence share the
        1/deg(dst) mean weight — so aggregation runs unweighted and the
        scale is applied once per pair row when copying PSUM->SBUF,
      * complete 320-dim pair rows are scattered to a DRAM table
        (slot = inv*PAD_N + local_node; slot 127 of each supertile is a
        dummy sink for padding edges).
  - Device phase 2 (per 128-node tile): PE-transpose the pair aggregates,
    then accumulate W_I @ agg0 + W_O @ agg1 + ent_emb @ W_S.T + biases
    in PSUM; plus the tiny time_emb update.
"""

import sys

sys.path.insert(0, "/opt/trn_rl_repo")

import numpy as np

import concourse.bacc as bacc
import concourse.bass as bass
import concourse.tile as tile
from concourse import mybir
from concourse.bass_utils import run_bass_kernel_spmd
from concourse.masks import make_identity

# Problem constants (hardcoded per spec; overridable for scaled-down tests).
CFG = dict(
    N=50000, E=500000, R=500, T=365,
    ENT=128, REL=128, TIM=64,
    NCORES=8,
)

P = 128
SUB = 5            # subtiles per supertile
ST_EDGES = SUB * P  # edges per supertile
ST_PAIRS = 127     # (dst, inv)-pair budget per supertile; slot 127 = dummy
MEGA = 8           # supertiles per megatile (bulk index loads)


def _pack_core(dst_c, inv_c, src_c, rel_c, tim_c, scale_c, base, pad_n):
    """Pack one core's (dst,inv)-sorted edges into supertiles."""
    ne = len(dst_c)
    if ne == 0:
        return (np.zeros((0, SUB, P), np.int32), np.zeros((0, SUB, P), np.float32),
                np.zeros((0, SUB, P), np.float32), np.zeros((0, SUB, P), np.float32),
                np.zeros((0, P), np.float32),
                np.zeros((0, P), np.int32), np.arange(2 * pad_n, dtype=np.int64))

    key = dst_c.astype(np.int64) * 2 + inv_c
    newg = np.r_[True, key[1:] != key[:-1]]
    gid = np.cumsum(newg) - 1
    ng = int(gid[-1]) + 1
    gsz = np.bincount(gid, minlength=ng)
    gdst = dst_c[newg].astype(np.int64)
    ginv = inv_c[newg].astype(np.int64)
    gscale = scale_c[newg]

    st_of_g = np.empty(ng, np.int64)
    col_of_g = np.empty(ng, np.int64)
    st_first_edge = [0]
    e_used = 0
    p_used = 0
    st = 0
    epos = 0
    for g in range(ng):
        s = int(gsz[g])
        if e_used + s > ST_EDGES or p_used + 1 > ST_PAIRS:
            st += 1
            st_first_edge.append(epos)
            e_used = 0
            p_used = 0
        st_of_g[g] = st
        col_of_g[g] = p_used
        e_used += s
        p_used += 1
        epos += s
    S = st + 1
    st_first_edge = np.asarray(st_first_edge, np.int64)

    st_e = st_of_g[gid]
    col_e = col_of_g[gid]
    pos_e = np.arange(ne, dtype=np.int64) - st_first_edge[st_e]
    fi = st_e * ST_EDGES + pos_e

    gent = np.zeros(S * ST_EDGES, np.int32)
    cola = np.full(S * ST_EDGES, float(ST_PAIRS), np.float32)  # dummies -> 127
    rela = np.zeros(S * ST_EDGES, np.float32)
    tima = np.zeros(S * ST_EDGES, np.float32)
    gent[fi] = src_c
    cola[fi] = col_e.astype(np.float32)
    rela[fi] = rel_c.astype(np.float32)
    tima[fi] = tim_c.astype(np.float32)

    pscale = np.zeros((S, P), np.float32)
    pscale[st_of_g, col_of_g] = gscale

    trash = (2 * pad_n + np.arange(P, dtype=np.int64))[None, :]
    scat = np.broadcast_to(trash, (S, P)).copy()
    scat[st_of_g, col_of_g] = ginv * pad_n + (gdst - base)

    present = np.zeros(2 * pad_n, bool)
    present[ginv * pad_n + (gdst - base)] = True
    missing = np.arange(2 * pad_n, dtype=np.int64)[~present]
    return (gent.reshape(S, SUB, P), cola.reshape(S, SUB, P),
            rela.reshape(S, SUB, P), tima.reshape(S, SUB, P),
            pscale, scat.astype(np.int32), missing)


def _host_prep(inputs, cfg):
    N, E, R, T = cfg["N"], cfg["E"], cfg["R"], cfg["T"]
    ENT, TIM = cfg["ENT"], cfg["TIM"]
    NC = cfg["NCORES"]
    NLOC = N // NC
    pad_n = ((NLOC + P - 1) // P) * P
    RCH = (R + P - 1) // P
    TCH = (T + P - 1) // P

    src = np.asarray(inputs["src"])
    dst = np.asarray(inputs["dst"])
    b_rel = np.asarray(inputs["b_rel"])
    time_idx = np.asarray(inputs["time_idx"])
    inv = np.asarray(inputs["inv"])

    cnt = np.bincount(dst, minlength=N).astype(np.float32)
    cnt_c = np.maximum(cnt, 1.0)
    scale_e = (1.0 / cnt_c)[dst].astype(np.float32)

    order = np.argsort(dst.astype(np.int64) * 2 + inv, kind="stable")
    dsts, invs, srcs, rels, tims, scls = (
        dst[order], inv[order], src[order], b_rel[order], time_idx[order],
        scale_e[order])

    bounds = np.searchsorted(dsts, np.arange(0, N + NLOC, NLOC))

    packs = []
    for c in range(NC):
        lo, hi = bounds[c], bounds[c + 1]
        packs.append(_pack_core(dsts[lo:hi], invs[lo:hi], srcs[lo:hi],
                                rels[lo:hi], tims[lo:hi], scls[lo:hi],
                                c * NLOC, pad_n))

    S_max = max(p[0].shape[0] for p in packs)
    S_max = ((S_max + MEGA - 1) // MEGA) * MEGA
    n_mega = S_max // MEGA
    zmax = max(len(p[6]) for p in packs)
    z_k = max(1, (zmax + P - 1) // P)
    trash_ids = 2 * pad_n + np.arange(z_k * P, dtype=np.int64) % P

    ent_emb = np.asarray(inputs["ent_emb"], np.float32)
    rel_emb = np.asarray(inputs["rel_emb"], np.float32)
    time_emb = np.asarray(inputs["time_emb"], np.float32)

    rel_pad = np.zeros((RCH * P, ENT), np.float32)
    rel_pad[:R] = rel_emb
    time_pad = np.zeros((TCH * P, TIM), np.float32)
    time_pad[:T] = time_emb

    W_I = np.asarray(inputs["W_I"], np.float32)
    W_O = np.asarray(inputs["W_O"], np.float32)
    W_S = np.asarray(inputs["W_S"], np.float32)
    W_T = np.asarray(inputs["W_T"], np.float32)
    COMP = W_I.shape[1]
    KCH = (COMP + P - 1) // P
    WI3 = np.zeros((P, KCH, ENT), np.float32)
    WO3 = np.zeros((P, KCH, ENT), np.float32)
    for k in range(KCH):
        kk = min(P, COMP - k * P)
        WI3[:kk, k] = W_I.T[k * P:k * P + kk]
        WO3[:kk, k] = W_O.T[k * P:k * P + kk]
    WST = np.ascontiguousarray(W_S.T)
    B3 = np.stack([np.asarray(inputs["b_I"], np.float32),
                   np.asarray(inputs["b_O"], np.float32),
                   np.asarray(inputs["b_S"], np.float32)])
    WTT = np.ascontiguousarray(W_T.T)
    bT = np.asarray(inputs["b_T"], np.float32)[None, :]
    timeT = np.ascontiguousarray(time_emb.T)

    c0 = np.bincount(dst[inv == 0], minlength=N).astype(np.float32)
    c1 = cnt - c0

    in_maps = []
    for c in range(NC):
        gent, cola, rela, tima, pscale, scat, missing = packs[c]
        S_c = gent.shape[0]
        padS = S_max - S_c
        if padS:
            gent = np.concatenate([gent, np.zeros((padS, SUB, P), np.int32)])
            cola = np.concatenate(
                [cola, np.full((padS, SUB, P), float(ST_PAIRS), np.float32)])
            rela = np.concatenate([rela, np.zeros((padS, SUB, P), np.float32)])
            tima = np.concatenate([tima, np.zeros((padS, SUB, P), np.float32)])
            pscale = np.concatenate([pscale, np.zeros((padS, P), np.float32)])
            trash = (2 * pad_n + np.arange(P, dtype=np.int64))[None, :]
            scat = np.concatenate(
                [scat, np.broadcast_to(trash, (padS, P)).astype(np.int32)])

        # [S, SUB, P] -> [n_mega, P, MEGA*SUB] with j = st_in_mega*SUB + sub
        def mg(a):
            return a.reshape(n_mega, MEGA * SUB, P).transpose(0, 2, 1)

        J = MEGA * SUB
        f32blk = np.concatenate(
            [mg(cola), mg(rela), mg(tima),
             pscale.reshape(n_mega, MEGA, P).transpose(0, 2, 1)], axis=-1)
        i32blk = np.concatenate(
            [mg(gent), scat.reshape(n_mega, MEGA, P).transpose(0, 2, 1)],
            axis=-1)

        zid = np.concatenate([missing, trash_ids])[:z_k * P]
        zid = zid.reshape(z_k, P).T

        lo = c * NLOC
        entT = np.zeros((ENT, pad_n), np.float32)
        entT[:, :NLOC] = ent_emb[lo:lo + NLOC].T
        beta = np.zeros((3, pad_n), np.float32)
        beta[0, :NLOC] = c0[lo:lo + NLOC] / cnt_c[lo:lo + NLOC]
        beta[1, :NLOC] = c1[lo:lo + NLOC] / cnt_c[lo:lo + NLOC]
        beta[2, :NLOC] = 1.0

        in_maps.append(dict(
            f32blk=np.ascontiguousarray(f32blk.astype(np.float32)),
            i32blk=np.ascontiguousarray(i32blk.astype(np.int32)),
            zids=np.ascontiguousarray(zid.astype(np.int32)),
            ent_tab=ent_emb, rel_pad=rel_pad, time_pad=time_pad,
            entT=entT, beta=beta,
            WI3=WI3.reshape(P, KCH * ENT), WO3=WO3.reshape(P, KCH * ENT),
            WST=WST, B3=B3, WTT=WTT, bT=bT, timeT=timeT,
        ))

    meta = dict(n_mega=n_mega, z_k=z_k, pad_n=pad_n, KCH=KCH, COMP=COMP,
                RCH=RCH, TCH=TCH)
    return in_maps, meta


def _build_program(meta, cfg):
    N, R, T = cfg["N"], cfg["R"], cfg["T"]
    ENT, TIM = cfg["ENT"], cfg["TIM"]
    NC = cfg["NCORES"]
    n_mega, z_k, pad_n = meta["n_mega"], meta["z_k"], meta["pad_n"]
    KCH, COMP, RCH, TCH = meta["KCH"], meta["COMP"], meta["RCH"], meta["TCH"]
    f32 = mybir.dt.float32
    i32 = mybir.dt.int32
    NT = pad_n // P
    J = MEGA * SUB                    # subtiles per megatile

    nc = bacc.Bacc("TRN2", target_bir_lowering=False, debug=False,
                   num_devices=NC)

    d_f32 = nc.dram_tensor("f32blk", [n_mega, P, 3 * J + MEGA], f32,
                           kind="ExternalInput").ap()
    d_i32 = nc.dram_tensor("i32blk", [n_mega, P, J + MEGA], i32,
                           kind="ExternalInput").ap()
    d_zids = nc.dram_tensor("zids", [P, z_k], i32, kind="ExternalInput").ap()
    d_ent = nc.dram_tensor("ent_tab", [N, ENT], f32,
                           kind="ExternalInput").ap()
    d_relp = nc.dram_tensor("rel_pad", [RCH * P, ENT], f32,
                            kind="ExternalInput").ap()
    d_timp = nc.dram_tensor("time_pad", [TCH * P, TIM], f32,
                            kind="ExternalInput").ap()
    d_entT = nc.dram_tensor("entT", [ENT, pad_n], f32,
                            kind="ExternalInput").ap()
    d_beta = nc.dram_tensor("beta", [3, pad_n], f32,
                            kind="ExternalInput").ap()
    d_WI3 = nc.dram_tensor("WI3", [P, KCH * ENT], f32,
                           kind="ExternalInput").ap()
    d_WO3 = nc.dram_tensor("WO3", [P, KCH * ENT], f32,
                           kind="ExternalInput").ap()
    d_WST = nc.dram_tensor("WST", [ENT, ENT], f32, kind="ExternalInput").ap()
    d_B3 = nc.dram_tensor("B3", [3, ENT], f32, kind="ExternalInput").ap()
    d_WTT = nc.dram_tensor("WTT", [TIM, TIM], f32, kind="ExternalInput").ap()
    d_bT = nc.dram_tensor("bT", [1, TIM], f32, kind="ExternalInput").ap()
    d_timeT = nc.dram_tensor("timeT", [TIM, T], f32,
                             kind="ExternalInput").ap()

    d_ent_out = nc.dram_tensor("ent_out", [pad_n, ENT], f32,
                               kind="ExternalOutput").ap()
    d_time_out = nc.dram_tensor("time_out", [T, TIM], f32,
                                kind="ExternalOutput").ap()
    d_tbl = nc.dram_tensor("tbl", [2 * pad_n + P, COMP], f32).ap()

    with tile.TileContext(nc) as tc:
        with tc.tile_pool(name="const", bufs=1) as cp:
            iota_i = cp.tile([P, RCH * P], i32)
            nc.gpsimd.iota(iota_i[:], pattern=[[1, RCH * P]], base=0,
                           channel_multiplier=0)
            iota_f = cp.tile([P, RCH * P], f32)
            nc.vector.tensor_copy(iota_f[:], iota_i[:])
            ident = cp.tile([P, P], f32)
            make_identity(nc, ident[:])
            rel_sb = cp.tile([P, RCH * P], f32)
            nc.sync.dma_start(
                out=rel_sb[:].rearrange("p (k c) -> p k c", k=RCH),
                in_=d_relp[:].rearrange("(k p) c -> p k c", p=P))
            tim_sb = cp.tile([P, TCH * TIM], f32)
            nc.sync.dma_start(
                out=tim_sb[:].rearrange("p (k c) -> p k c", k=TCH),
                in_=d_timp[:].rearrange("(k p) c -> p k c", p=P))
            wi = cp.tile([P, KCH * ENT], f32)
            nc.sync.dma_start(out=wi[:], in_=d_WI3[:])
            wo = cp.tile([P, KCH * ENT], f32)
            nc.sync.dma_start(out=wo[:], in_=d_WO3[:])
            wst = cp.tile([ENT, ENT], f32)
            nc.sync.dma_start(out=wst[:], in_=d_WST[:])
            b3 = cp.tile([3, ENT], f32)
            nc.sync.dma_start(out=b3[:], in_=d_B3[:])
            wtt = cp.tile([TIM, TIM], f32)
            nc.sync.dma_start(out=wtt[:], in_=d_WTT[:])
            bt = cp.tile([1, TIM], f32)
            nc.sync.dma_start(out=bt[:], in_=d_bT[:])
            ttl = cp.tile([TIM, T], f32)
            nc.sync.dma_start(out=ttl[:], in_=d_timeT[:])
            ones = cp.tile([1, P], f32)
            nc.vector.memset(ones[:], 1.0)
            zt = cp.tile([P, COMP], f32)
            nc.vector.memset(zt[:], 0.0)

            # zero the (node, inv) slots that receive no edges
            zid_t = cp.tile([P, z_k], i32)
            nc.sync.dma_start(out=zid_t[:], in_=d_zids[:])
            for z in range(z_k):
                nc.gpsimd.indirect_dma_start(
                    out=d_tbl[:, :],
                    out_offset=bass.IndirectOffsetOnAxis(
                        ap=zid_t[:, z:z + 1], axis=0),
                    in_=zt[:], in_offset=None)

            # ---------------- phase 1: gather + aggregate + scatter --------
            with (tc.tile_pool(name="p1", bufs=3) as p1,
                  tc.tile_pool(name="p1g", bufs=6) as p1g,
                  tc.tile_pool(name="p1s", bufs=3) as p1s,
                  tc.tile_pool(name="ps1", bufs=2, space="PSUM") as ps1):
                for m in range(n_mega):
                    ft = p1.tile([P, 3 * J + MEGA], f32, tag="ft")
                    nc.sync.dma_start(out=ft[:], in_=d_f32[m])
                    it = p1.tile([P, J + MEGA], i32, tag="it")
                    nc.sync.dma_start(out=it[:], in_=d_i32[m])
                    for s in range(MEGA):
                        pagg = ps1.tile([P, COMP], f32, space="PSUM",
                                        tag="pagg")
                        phtr = ps1.tile([P, RCH * P], f32, space="PSUM",
                                        tag="phtr")
                        phtt = ps1.tile([P, TCH * P], f32, space="PSUM",
                                        tag="phtt")
                        ohp = p1.tile([P, SUB * P], f32, tag="ohp")
                        ohr = p1.tile([P, SUB * RCH * P], f32, tag="ohr")
                        oht = p1.tile([P, SUB * TCH * P], f32, tag="oht")
                        for b in range(SUB):
                            j = s * SUB + b
                            ent_g = p1g.tile([P, ENT], f32, tag="entg")
                            nc.gpsimd.indirect_dma_start(
                                out=ent_g[:], out_offset=None, in_=d_ent[:, :],
                                in_offset=bass.IndirectOffsetOnAxis(
                                    ap=it[:, j:j + 1], axis=0))
                            nc.vector.tensor_scalar(
                                out=ohp[:, b * P:(b + 1) * P],
                                in0=iota_f[:, :P],
                                scalar1=ft[:, j:j + 1], scalar2=None,
                                op0=mybir.AluOpType.is_equal)
                            nc.vector.tensor_scalar(
                                out=ohr[:, b * RCH * P:(b + 1) * RCH * P],
                                in0=iota_f[:],
                                scalar1=ft[:, J + j:J + j + 1], scalar2=None,
                                op0=mybir.AluOpType.is_equal)
                            nc.vector.tensor_scalar(
                                out=oht[:, b * TCH * P:(b + 1) * TCH * P],
                                in0=iota_f[:, :TCH * P],
                                scalar1=ft[:, 2 * J + j:2 * J + j + 1],
                                scalar2=None,
                                op0=mybir.AluOpType.is_equal)
                            nc.tensor.matmul(pagg[:, P:2 * P],
                                             lhsT=ohp[:, b * P:(b + 1) * P],
                                             rhs=ent_g[:], start=(b == 0),
                                             stop=(b == SUB - 1))
                        for k in range(RCH):
                            for b in range(SUB):
                                nc.tensor.matmul(
                                    phtr[:, k * P:(k + 1) * P],
                                    lhsT=ohr[:, (b * RCH + k) * P:
                                             (b * RCH + k + 1) * P],
                                    rhs=ohp[:, b * P:(b + 1) * P],
                                    start=(b == 0), stop=(b == SUB - 1))
                        for k in range(TCH):
                            for b in range(SUB):
                                nc.tensor.matmul(
                                    phtt[:, k * P:(k + 1) * P],
                                    lhsT=oht[:, (b * TCH + k) * P:
                                             (b * TCH + k + 1) * P],
                                    rhs=ohp[:, b * P:(b + 1) * P],
                                    start=(b == 0), stop=(b == SUB - 1))
                        htr = p1.tile([P, RCH * P], f32, tag="htr")
                        nc.vector.tensor_copy(htr[:], phtr[:])
                        htt = p1.tile([P, TCH * P], f32, tag="htt")
                        nc.vector.tensor_copy(htt[:], phtt[:])
                        for k in range(RCH):
                            nc.tensor.matmul(
                                pagg[:, 0:P],
                                lhsT=htr[:, k * P:(k + 1) * P],
                                rhs=rel_sb[:, k * P:(k + 1) * P],
                                start=(k == 0), stop=(k == RCH - 1))
                        for k in range(TCH):
                            nc.tensor.matmul(
                                pagg[:, 2 * P:2 * P + TIM],
                                lhsT=htt[:, k * P:(k + 1) * P],
                                rhs=tim_sb[:, k * TIM:(k + 1) * TIM],
                                start=(k == 0), stop=(k == TCH - 1))
                        staging = p1s.tile([P, COMP], f32, tag="staging")
                        nc.vector.tensor_scalar(
                            out=staging[:], in0=pagg[:],
                            scalar1=ft[:, 3 * J + s:3 * J + s + 1],
                            scalar2=None, op0=mybir.AluOpType.mult)
                        nc.gpsimd.indirect_dma_start(
                            out=d_tbl[:, :],
                            out_offset=bass.IndirectOffsetOnAxis(
                                ap=it[:, J + s:J + s + 1], axis=0),
                            in_=staging[:], in_offset=None)

            # ---------------- phase 2: per-node dense update ---------------
            with (tc.tile_pool(name="p2", bufs=2) as p2,
                  tc.tile_pool(name="ps2", bufs=2, space="PSUM") as ps2,
                  tc.tile_pool(name="ps2o", bufs=2, space="PSUM") as ps2o):
                for t in range(NT):
                    a0 = p2.tile([P, COMP], f32, tag="a0")
                    nc.sync.dma_start(out=a0[:], in_=d_tbl[t * P:(t + 1) * P, :])
                    a1 = p2.tile([P, COMP], f32, tag="a1")
                    nc.sync.dma_start(
                        out=a1[:], in_=d_tbl[pad_n + t * P:pad_n + (t + 1) * P, :])
                    et = p2.tile([ENT, P], f32, tag="et")
                    nc.sync.dma_start(out=et[:], in_=d_entT[:, t * P:(t + 1) * P])
                    bet = p2.tile([3, P], f32, tag="bet")
                    nc.sync.dma_start(out=bet[:], in_=d_beta[:, t * P:(t + 1) * P])
                    pso = ps2o.tile([P, ENT], f32, space="PSUM", tag="pso")
                    first = True
                    for a, w in ((a0, wi), (a1, wo)):
                        for k in range(KCH):
                            kk = min(P, COMP - k * P)
                            pt = ps2.tile([P, P], f32, space="PSUM", tag="pt")
                            nc.tensor.transpose(
                                out=pt[:kk, :], in_=a[:, k * P:k * P + kk],
                                identity=ident[:])
                            aT = p2.tile([P, P], f32, tag="aT")
                            nc.vector.tensor_copy(aT[:kk, :], pt[:kk, :])
                            nc.tensor.matmul(
                                pso[:], lhsT=aT[:kk, :],
                                rhs=w[:kk, k * ENT:(k + 1) * ENT],
                                start=first, stop=False)
                            first = False
                    nc.tensor.matmul(pso[:], lhsT=bet[:3, :], rhs=b3[:3, :],
                                     start=False, stop=False)
                    nc.tensor.matmul(pso[:], lhsT=et[:], rhs=wst[:],
                                     start=False, stop=True)
                    osb = p2.tile([P, ENT], f32, tag="osb")
                    nc.vector.tensor_copy(osb[:], pso[:])
                    nc.sync.dma_start(out=d_ent_out[t * P:(t + 1) * P, :],
                                      in_=osb[:])

                # time update
                mo = 0
                while mo < T:
                    msz = min(P, T - mo)
                    pst = ps2.tile([P, TIM], f32, space="PSUM", tag="pst")
                    nc.tensor.matmul(pst[:msz, :], lhsT=ttl[:, mo:mo + msz],
                                     rhs=wtt[:], start=True, stop=False)
                    nc.tensor.matmul(pst[:msz, :], lhsT=ones[:1, :msz],
                                     rhs=bt[:1, :], start=False, stop=True)
                    tsb = p2.tile([P, TIM], f32, tag="tsb")
                    nc.vector.tensor_copy(tsb[:msz, :], pst[:msz, :])
                    nc.sync.dma_start(out=d_time_out[mo:mo + msz, :],
                                      in_=tsb[:msz, :])
                    mo += msz

    nc.compile()
    return nc


def kernel(**inputs):
    cfg = CFG
    in_maps, meta = _host_prep(inputs, cfg)
    nc = _build_program(meta, cfg)
    res = run_bass_kernel_spmd(nc, in_maps, list(range(cfg["NCORES"])))
    NLOC = cfg["N"] // cfg["NCORES"]
    ent_new = np.concatenate(
        [res.results[c]["ent_out"][:NLOC] for c in range(cfg["NCORES"])])
    time_new = res.results[0]["time_out"]
    return ent_new, time_new
